# revision 31
# baseline (speedup 1.0000x reference)
"""GNN message passing (GraphConvolution) on 8 TRN2 NeuronCores.

reference:
    support = x @ W                                   # [N, H]
    msgs    = support[edge_src] * edge_w[:, None]     # [E, H]
    agg     = segment_sum(msgs, edge_dst, N)          # [N, H]
    out     = relu(agg + b)

Strategy (dst-node 1D sharding; sharded support build + AllGather):
  - Core c owns dst nodes [c*NPC, (c+1)*NPC).
  - Phase 1 sharded: core c computes support rows [c*12800, (c+1)*12800)
    (bf16), one 8-way AllGather replicates the full table to every core.
  - Phase 2: edges bucketed by (dst subtile of 128, src run of 32768);
    per (supertile=512, run) call: gpsimd dma_gather of the bucket's
    support rows (128-row chunks, idx int16 per run), weighted one-hot
    indicator built on DVE (is_equal vs iota, then *ew), one TensorE
    matmul per 128-edge chunk accumulating psum[h, dst-subtile].
  - PSUM: one [128,512] bank per supertile, held across all 4 runs for
    a group of 8 supertiles at a time (start on the supertile's first
    chunk, stop on its last; the start marks the whole 2KB zero region).
  - Gathers spread over 4 SWDGE queues (greedy balance) so all 4 Q7
    core pairs generate descriptors concurrently (the bottleneck).
  - Optionally (GNN_EWSC percent of calls) the *ew multiply moves to
    ScalarE as a per-chunk Copy-activation with per-partition scale on
    the gathered tile, relieving DVE.
  - Epilogue relu(psum + b) on ScalarE -> outT [H, NPC] -> host transpose.
"""

import math
import os

import ml_dtypes
import numpy as np

import concourse.bass as bass
import concourse.mybir as mybir
import concourse.tile as tile
from concourse import bacc
from concourse.bass_utils import run_bass_kernel_spmd
from concourse.library_config import mlp as _mlp_lib

BF16 = ml_dtypes.bfloat16
SUB = 128
SUPER = 512
CHUNK = 32768

N_NODES = 100000
NFEAT = 256
NHID = 128
N_CORES = 8
NPC = N_NODES // N_CORES  # 12500
NPAD = 102400  # 8 * 12800, multiple of 512
SHARD = NPAD // N_CORES  # 12800 support rows computed per core
N_RUNS = 4
SGROUP = 8  # supertiles per psum-resident group
USE_AG = bool(int(os.environ.get("GNN_AG", "1")))
EWSC = int(os.environ.get("GNN_EWSC", "0"))  # % of calls with ew-mult on ScalarE


def _ceil_div(a, b):
    return (a + b - 1) // b


def _run_rows(r):
    r0 = r * CHUNK
    r1 = min((r + 1) * CHUNK, NPAD)
    return r0, r1


def prepare(x, edge_src, edge_dst, edge_w, W, b):
    n_nodes, nfeat = x.shape
    nhid = W.shape[1]
    assert (n_nodes, nfeat, nhid) == (N_NODES, NFEAT, NHID)
    n_super = _ceil_div(NPC, SUPER)  # 25
    max_so = SUPER // SUB  # 4

    src = np.asarray(edge_src).astype(np.int64)
    dst = np.asarray(edge_dst).astype(np.int64)
    ew = np.asarray(edge_w).astype(np.float32)

    core_of = dst // NPC
    per_core = []
    counts = np.zeros((N_CORES, n_super, max_so, N_RUNS), np.int64)
    for c in range(N_CORES):
        m = core_of == c
        s_c = src[m]
        d_c = dst[m] - c * NPC
        w_c = ew[m]
        S_c = d_c >> 9
        so_c = (d_c >> 7) - 4 * S_c
        r_c = s_c >> 15
        key = ((S_c * N_RUNS + r_c) * max_so) + so_c
        order = np.argsort(key, kind="stable")
        s_c, d_c, w_c, key = s_c[order], d_c[order], w_c[order], key[order]
        S_o, so_o, r_o = S_c[order], so_c[order], r_c[order]
        np.add.at(counts[c], (S_o, so_o, r_o), 1)
        per_core.append((s_c, d_c, w_c, key))

    # g_tab[S, so, r] = chunks for that bucket (maxed over cores)
    g_tab = _ceil_div(counts.max(axis=0), 128)
    for S in range(n_super):
        n_sub_here = _ceil_div(min(SUPER, NPC - S * SUPER), SUB)
        g_tab[S, 0, 0] = max(g_tab[S, 0, 0], 1)  # start anchor
        g_tab[S, n_sub_here - 1, 3] = max(g_tab[S, n_sub_here - 1, 3], 1)  # stop

    # program order: groups of SGROUP supertiles, runs inside, supertiles inside
    calls = []
    chunk_off = 0
    groups = [
        list(range(g, min(g + SGROUP, n_super))) for g in range(0, n_super, SGROUP)
    ]
    for grp in groups:
        for r in range(N_RUNS):
            for S in grp:
                wS = min(SUPER, NPC - S * SUPER)
                n_sub_here = _ceil_div(wS, SUB)
                meta = []
                for so in range(n_sub_here):
                    meta.extend([so] * int(g_tab[S, so, r]))
                if not meta:
                    continue
                calls.append(
                    dict(
                        r=r,
                        S=S,
                        n_chunks=len(meta),
                        chunk_off=chunk_off,
                        meta=meta,
                        first=(r == 0),
                        last=(r == 3),
                    )
                )
                chunk_off += len(meta)
    nchunk = chunk_off
    e_pad = nchunk * 128
    gmax = max(cl["n_chunks"] for cl in calls)

    # start/stop flags: start on S's first chunk (r==0 first call for S),
    # stop on S's last chunk (r==3 last call for S). Calls for S are in
    # r order, so 'first'/'last' flags above identify them.
    # greedy queue balance + ew-engine split
    qload = [0, 0, 0, 0]
    n_sc = 0
    for i, cl in enumerate(calls):
        q = min(range(4), key=lambda k: qload[k])
        cl["queue"] = q
        qload[q] += cl["n_chunks"]
        cl["ew_sc"] = (i * EWSC) // 100 != ((i + 1) * EWSC) // 100
        n_sc += int(cl["ew_sc"])

    in_maps = []
    xT = np.zeros((NFEAT, NPAD), BF16)
    xT[:, :N_NODES] = np.asarray(x, np.float32).T.astype(BF16)
    w_bf = np.ascontiguousarray(np.asarray(W, np.float32).astype(BF16))
    bias = np.asarray(b, np.float32).reshape(nhid, 1).copy()
    iota = np.tile(np.arange(SUB, dtype=np.float32).astype(BF16)[None, :], (128, 1))

    for c in range(N_CORES):
        s_c, d_c, w_c, key = per_core[c]
        idx_pad = np.zeros(e_pad, np.int16)
        dl_pad = np.zeros(e_pad, BF16)
        ew_pad = np.zeros(e_pad, np.float32)
        uniq, first = np.unique(key, return_index=True)
        uniq = uniq.tolist()
        first = first.tolist()
        total = len(s_c)

        import bisect

        def seg(S, r, so):
            k = (S * N_RUNS + r) * max_so + so
            i = bisect.bisect_left(uniq, k)
            if i >= len(uniq) or uniq[i] != k:
                return 0, 0
            a = first[i]
            bnd = first[i + 1] if i + 1 < len(uniq) else total
            return a, bnd

        for cl in calls:
            r, S = cl["r"], cl["S"]
            run0, _ = _run_rows(r)
            pos = cl["chunk_off"] * 128
            prev_so = -1
            for so in sorted(set(cl["meta"])):
                a, bnd = seg(S, r, so)
                n = bnd - a
                capn = cl["meta"].count(so) * 128
                assert n <= capn, (c, S, r, so, n, capn)
                p0 = cl["chunk_off"] * 128 + cl["meta"].index(so) * 128
                idx_pad[p0 : p0 + n] = (s_c[a:bnd] - run0).astype(np.int16)
                dl_pad[p0 : p0 + n] = (d_c[a:bnd] & (SUB - 1)).astype(BF16)
                ew_pad[p0 : p0 + n] = w_c[a:bnd].astype(np.float32)

        in_maps.append(
            dict(
                xT=np.ascontiguousarray(xT[:, c * SHARD : (c + 1) * SHARD])
                if USE_AG
                else xT,
                wmat=w_bf,
                bias=bias,
                iota=iota,
                idx=np.ascontiguousarray(np.tile(idx_pad.reshape(-1, 16).T, (8, 1))),
                dstloc=np.ascontiguousarray(dl_pad.reshape(-1, 128).T),
                ew=np.ascontiguousarray(ew_pad.reshape(-1, 128).T),
            )
        )

    cfg = dict(
        nfeat=NFEAT,
        nhid=NHID,
        n_cores=N_CORES,
        npc=NPC,
        n_super=n_super,
        nchunk=nchunk,
        e_pad=e_pad,
        gmax=gmax,
        calls=calls,
        groups=groups,
    )
    return cfg, in_maps


def build_bass(cfg):
    F, H = cfg["nfeat"], cfg["nhid"]
    KC = F // 128
    n_super = cfg["n_super"]
    XBLK = 512

    nc = bacc.Bacc(
        "TRN2",
        target_bir_lowering=False,
        debug=False,
        enable_asserts=True,
        num_devices=cfg["n_cores"],
        num_swdge_queues=4,
    )
    f32, bf16, i16 = mybir.dt.float32, mybir.dt.bfloat16, mybir.dt.int16
    xT = nc.dram_tensor(
        "xT", [F, SHARD if USE_AG else NPAD], bf16, kind="ExternalInput"
    )
    wmat = nc.dram_tensor("wmat", [F, H], bf16, kind="ExternalInput")
    bias = nc.dram_tensor("bias", [H, 1], f32, kind="ExternalInput")
    iota = nc.dram_tensor("iota", [128, SUB], bf16, kind="ExternalInput")
    idx = nc.dram_tensor("idx", [128, cfg["e_pad"] // 16], i16, kind="ExternalInput")
    dstloc = nc.dram_tensor("dstloc", [128, cfg["nchunk"]], bf16, kind="ExternalInput")
    ew = nc.dram_tensor("ew", [128, cfg["nchunk"]], f32, kind="ExternalInput")
    outT = nc.dram_tensor("outT", [H, cfg["npc"]], f32, kind="ExternalOutput")

    AF = mybir.ActivationFunctionType
    ALU = mybir.AluOpType
    rg = [list(range(cfg["n_cores"]))]

    with tile.TileContext(nc) as tc:
        with (
            tc.tile_pool(name="dram", bufs=1, space="DRAM") as dpool,
            tc.tile_pool(name="const", bufs=1) as cpool,
            tc.tile_pool(name="xt", bufs=3) as xpool,
            tc.tile_pool(name="sup", bufs=3) as spool,
            tc.tile_pool(name="gath", bufs=8) as gpool,
            tc.tile_pool(name="gtw", bufs=4) as wpool,
            tc.tile_pool(name="ind", bufs=6) as ipool,
            tc.tile_pool(name="meta", bufs=8) as mpool,
            tc.tile_pool(name="outb", bufs=2) as opool,
            tc.tile_pool(name="ps", bufs=8, space="PSUM") as ppool,
        ):
            nc.gpsimd.load_library(_mlp_lib)
            w_sb = cpool.tile([128, KC, H], bf16)
            nc.sync.dma_start(
                out=w_sb[:], in_=wmat.ap().rearrange("(c k) h -> k c h", k=128)
            )
            bias_sb = cpool.tile([H, 1], f32)
            nc.sync.dma_start(out=bias_sb[:], in_=bias.ap())
            iota_sb = cpool.tile([128, SUB], bf16)
            nc.sync.dma_start(out=iota_sb[:], in_=iota.ap())

            if USE_AG:
                ag_in = dpool.tile([SHARD, H], bf16, name="agin", tag="agin")
            support = dpool.tile([NPAD, H], bf16, name="supp", tag="supp")

            # ---- phase 1: support = x @ W (sharded when USE_AG) ----
            n_blocks = (SHARD if USE_AG else NPAD) // XBLK
            for blk in range(n_blocks):
                xts = []
                for kc in range(KC):
                    xt = xpool.tile([128, XBLK], bf16, tag=f"xt{kc}")
                    nc.sync.dma_start(
                        out=xt[:],
                        in_=xT.ap()[
                            kc * 128 : (kc + 1) * 128, blk * XBLK : (blk + 1) * XBLK
                        ],
                    )
                    xts.append(xt)
                st = spool.tile([128, XBLK], bf16)
                ps1 = ppool.tile([128, XBLK], f32, tag="agg")
                n_col = XBLK // 128
                for i in range(n_col):
                    col = i * 128
                    for kc in range(KC):
                        nc.tensor.matmul(
                            ps1[:, col : col + 128],
                            xts[kc][:, col : col + 128],
                            w_sb[:, kc, :],
                            start=(i == 0 and kc == 0),
                            stop=(i == n_col - 1 and kc == KC - 1),
                        )
                nc.scalar.activation(out=st[:], in_=ps1[:], func=AF.Copy)
                wr_dst = ag_in if USE_AG else support
                nc.sync.dma_start(
                    out=wr_dst[blk * XBLK : (blk + 1) * XBLK, :].rearrange(
                        "(i p) h -> p i h", p=128
                    ),
                    in_=st[:].rearrange("p (i h) -> p i h", h=H),
                )
            if USE_AG:
                nc.gpsimd.collective_compute(
                    "AllGather",
                    ALU.bypass,
                    replica_groups=rg,
                    ins=[ag_in.opt()],
                    outs=[support.opt()],
                )

            # ---- phase 2: per (supertile, run) gather + one-hot matmuls ----
            # hoist the num_idxs registers (few distinct values) so the
            # per-call MOVE doesn't WAR-serialize the gather stream
            lregs = {}
            for cl in cfg["calls"]:
                L = cl["n_chunks"] * 128
                if L not in lregs:
                    lregs[L] = nc.gpsimd.to_reg(L)
            pss = {}
            for cl in cfg["calls"]:
                r, S = cl["r"], cl["S"]
                run0, run1 = _run_rows(r)
                Gc = cl["n_chunks"]
                L = Gc * 128
                c0 = cl["chunk_off"] * 8
                idxt = mpool.tile([128, L // 16], i16, tag="idx")
                nc.sync.dma_start(out=idxt[:], in_=idx.ap()[:, c0 : c0 + L // 16])
                dlt = mpool.tile([128, Gc], bf16, tag="dl")
                nc.sync.dma_start(
                    out=dlt[:],
                    in_=dstloc.ap()[:, cl["chunk_off"] : cl["chunk_off"] + Gc],
                )
                ewt = mpool.tile([128, Gc], f32, tag="ew")
                nc.sync.dma_start(
                    out=ewt[:], in_=ew.ap()[:, cl["chunk_off"] : cl["chunk_off"] + Gc]
                )
                gt = gpool.tile([128, cfg["gmax"], H], bf16, tag="gt")
                nc.gpsimd.dma_gather(
                    gt[:, :Gc, :],
                    support[run0:run1, :],
                    idxt[:],
                    L,
                    lregs[L],
                    H,
                    single_packet=bool(int(os.environ.get("GNN_SP", "0"))),
                    queue_num=cl["queue"],
                )
                if cl["first"]:
                    pss[S] = ppool.tile([128, SUPER], f32, tag="agg", name=f"psS{S}")
                ps = pss[S]
                ind = ipool.tile([128, Gc, SUB], bf16, tag="ind")
                nc.vector.tensor_tensor(
                    out=ind[:],
                    in0=iota_sb[:][:, None, :].to_broadcast([128, Gc, SUB]),
                    in1=dlt[:][:, :, None].to_broadcast([128, Gc, SUB]),
                    op=ALU.is_equal,
                )
                if cl["ew_sc"]:
                    # ScalarE applies the per-edge weight to the indicator
                    # (keeps gt consumed only by fast matmuls, so the
                    # gather pipeline isn't gated on ScalarE)
                    indw = wpool.tile([128, cfg["gmax"], SUB], bf16, tag="indw")
                    for j in range(Gc):
                        nc.scalar.activation(
                            out=indw[:, j, :],
                            in_=ind[:, j, :],
                            func=AF.Copy,
                            scale=ewt[:, j : j + 1],
                        )
                    mm_ind = indw
                else:
                    nc.vector.tensor_tensor(
                        out=ind[:],
                        in0=ind[:],
                        in1=ewt[:][:, :, None].to_broadcast([128, Gc, SUB]),
                        op=ALU.mult,
                    )
                    mm_ind = ind
                mm_in = gt
                first_of_S = cl["first"]
                last_of_S = cl["last"]
                for j, so in enumerate(cl["meta"]):
                    nc.tensor.matmul(
                        ps[:, so * SUB : (so + 1) * SUB],
                        mm_in[:, j, :],
                        mm_ind[:, j, :],
                        start=(first_of_S and j == 0),
                        stop=(last_of_S and j == Gc - 1),
                    )
                if last_of_S:
                    wS = min(SUPER, cfg["npc"] - S * SUPER)
                    ob = opool.tile([H, SUPER], f32)
                    nc.scalar.activation(
                        out=ob[:, :wS],
                        in_=ps[:, :wS],
                        func=AF.Relu,
                        bias=bias_sb[:],
                        scale=1.0,
                    )
                    nc.sync.dma_start(
                        out=outT.ap()[:, S * SUPER : S * SUPER + wS], in_=ob[:, :wS]
                    )
                    del pss[S]
    nc.compile()
    return nc


def kernel(x, edge_src, edge_dst, edge_w, W, b):
    x = np.asarray(x)
    cfg, in_maps = prepare(x, edge_src, edge_dst, edge_w, W, b)
    nc = build_bass(cfg)
    want_trace = bool(int(os.environ.get("GNN_TRACE", "0")))
    core_ids = list(range(cfg["n_cores"]))
    if want_trace:
        try:
            res = run_bass_kernel_spmd(nc, in_maps, core_ids=core_ids, trace=True)
        except Exception as e:
            print(f"traced run failed ({e}); retrying without trace")
            res = run_bass_kernel_spmd(nc, in_maps, core_ids=core_ids, trace=False)
    else:
        res = run_bass_kernel_spmd(nc, in_maps, core_ids=core_ids, trace=False)
    kernel.last_result = res
    out = np.concatenate([r["outT"].T for r in res.results], axis=0)
    return np.ascontiguousarray(out).astype(np.float32)


kernel.last_result = None


# revision 32
# speedup vs baseline: 1.2694x; 1.2694x over previous
"""GNN message passing (GraphConvolution) on 8 TRN2 NeuronCores.

reference:
    support = x @ W                                   # [N, H]
    msgs    = support[edge_src] * edge_w[:, None]     # [E, H]
    agg     = segment_sum(msgs, edge_dst, N)          # [N, H]
    out     = relu(agg + b)

Strategy (dst-node 1D sharding, no collectives):
  - Core c owns dst nodes [c*NPC, (c+1)*NPC).
  - Every core computes the full support table (x@W, bf16) into its own
    DRAM: xT is staged bf16/pre-transposed on host, matmul on TensorE.
  - Host routes edges: per core, edges are bucketed by
    (dst subtile of 128, src chunk of 32768) and sorted; each bucket is
    padded to a multiple of 128 "edges" (idx=0, w=0). Bucket sizes are
    maxed over cores so a single SPMD NEFF works for all 8 cores.
  - Device gathers support rows with gpsimd.dma_gather (int16 indices,
    hence the 32768-row src chunking), builds a weighted one-hot
    indicator S[e, d] = w_e * (dstloc_e == d) on VectorE, and reduces
    each 128-edge chunk with one TensorE matmul accumulating in PSUM:
        psum[h, d] += gathered[e, h]^T-contract-e S[e, d]
  - Epilogue: ScalarE relu(psum + b) -> outT [H, NPC] -> host transpose.
"""

import math
import os

import ml_dtypes
import numpy as np

import concourse.bass as bass
import concourse.mybir as mybir
import concourse.tile as tile
from concourse import bacc
from concourse.bass_utils import run_bass_kernel_spmd
from concourse.library_config import mlp as _mlp_lib

BF16 = ml_dtypes.bfloat16
SUB = 128  # dst nodes per PSUM column block (one-hot width)
PSUM_COLS = 512  # PSUM bank tile free dim = subtiles-per-supertile * SUB


def _ceil_div(a, b):
    return (a + b - 1) // b


def prepare(x, edge_src, edge_dst, edge_w, W, b, *, n_cores=8, chunk=32768, xblk=2048):
    """Host-side sharding/routing. Returns (cfg, in_maps)."""
    n_nodes, nfeat = x.shape
    nhid = W.shape[1]
    assert n_nodes % n_cores == 0
    npc = n_nodes // n_cores
    assert chunk & (chunk - 1) == 0 and SUB & (SUB - 1) == 0
    npad = _ceil_div(n_nodes, xblk) * xblk
    n_sub = _ceil_div(npc, SUB)
    sps = PSUM_COLS // SUB  # subtiles per supertile
    n_super = _ceil_div(n_sub, sps)
    n_runs = _ceil_div(n_nodes, chunk)
    log2_chunk = chunk.bit_length() - 1

    src = np.asarray(edge_src).astype(np.int64)
    dst = np.asarray(edge_dst).astype(np.int64)
    ew = np.asarray(edge_w).astype(np.float32)

    core_of = dst // npc
    per_core = []
    counts = np.zeros((n_cores, n_sub, n_runs), np.int64)
    for c in range(n_cores):
        m = core_of == c
        s_c = src[m]
        d_c = dst[m] - c * npc
        w_c = ew[m]
        sub_c = d_c >> 7
        run_c = s_c >> log2_chunk
        key = sub_c * n_runs + run_c
        order = np.argsort(key, kind="stable")
        s_c, d_c, w_c, key = s_c[order], d_c[order], w_c[order], key[order]
        cnt = np.bincount(key, minlength=n_sub * n_runs).reshape(n_sub, n_runs)
        counts[c] = cnt
        seg_start = np.zeros(n_sub * n_runs + 1, np.int64)
        np.cumsum(cnt.reshape(-1), out=seg_start[1:])
        per_core.append((s_c, d_c, w_c, seg_start))

    g_tab = np.maximum(_ceil_div(counts.max(axis=0), 128), 0).astype(np.int64)
    g_tab[:, 0] = np.maximum(g_tab[:, 0], 1)  # every subtile gets >=1 chunk

    # Static call/chunk structure, in device program order: (S, r, s).
    supers = []
    chunk_off = 0
    seen = np.zeros(n_sub, np.int64)  # chunks placed so far per subtile
    total = g_tab.sum(axis=1)  # total chunks per subtile
    for S in range(n_super):
        subs = list(range(S * sps, min(S * sps + sps, n_sub)))
        calls = []
        for r in range(n_runs):
            n_chunks = int(sum(g_tab[s, r] for s in subs))
            if n_chunks == 0:
                continue
            sub_local, c_start, c_stop = [], [], []
            for s in subs:
                for _ in range(int(g_tab[s, r])):
                    sub_local.append(s - S * sps)
                    c_start.append(seen[s] == 0)
                    c_stop.append(seen[s] == total[s] - 1)
                    seen[s] += 1
            calls.append(
                dict(
                    r=r,
                    n_chunks=n_chunks,
                    chunk_off=chunk_off,
                    sub_local=sub_local,
                    start=c_start,
                    stop=c_stop,
                    row0=r * chunk,
                    row1=min((r + 1) * chunk, npad),
                )
            )
            chunk_off += n_chunks
        w_cols = min(PSUM_COLS, npc - S * PSUM_COLS)
        supers.append(dict(calls=calls, w=w_cols))
    nchunk = int(chunk_off)
    e_pad = nchunk * 128

    # real (per-segment) edge offsets, following the same (S, r, s) order
    seg_edge_off = {}
    pos = 0
    for S in range(n_super):
        subs = list(range(S * sps, min(S * sps + sps, n_sub)))
        for r in range(n_runs):
            for s in subs:
                if g_tab[s, r] == 0:
                    continue
                seg_edge_off[(s, r)] = pos
                pos += int(g_tab[s, r]) * 128
    assert pos == e_pad

    in_maps = []
    xT = np.zeros((nfeat, npad), BF16)
    xT[:, :n_nodes] = np.asarray(x, np.float32).T.astype(BF16)
    w_bf = np.ascontiguousarray(np.asarray(W, np.float32).astype(BF16))
    bias = np.asarray(b, np.float32).reshape(nhid, 1).copy()
    iota = np.tile(np.arange(SUB, dtype=np.float32).astype(BF16)[None, :], (128, 1))
    for c in range(n_cores):
        s_c, d_c, w_c, seg_start = per_core[c]
        idx_pad = np.zeros(e_pad, np.int16)
        dl_pad = np.zeros(e_pad, np.float32)
        ww_pad = np.zeros(e_pad, np.float32)
        for (s, r), off in seg_edge_off.items():
            k = s * n_runs + r
            a, bnd = seg_start[k], seg_start[k + 1]
            n = bnd - a
            if n == 0:
                continue
            idx_pad[off : off + n] = (s_c[a:bnd] & (chunk - 1)).astype(np.int16)
            dl_pad[off : off + n] = (d_c[a:bnd] & (SUB - 1)).astype(np.float32)
            ww_pad[off : off + n] = w_c[a:bnd]
        in_maps.append(
            dict(
                xT=xT,
                wmat=w_bf,
                bias=bias,
                iota=iota,
                idx=np.ascontiguousarray(np.tile(idx_pad.reshape(-1, 16).T, (8, 1))),
                dstloc=np.ascontiguousarray(dl_pad.reshape(-1, 128).T.astype(BF16)),
                ew=np.ascontiguousarray(ww_pad.reshape(-1, 128).T.astype(BF16)),
            )
        )

    cfg = dict(
        n_nodes=n_nodes,
        npad=npad,
        nfeat=nfeat,
        nhid=nhid,
        n_cores=n_cores,
        npc=npc,
        chunk=chunk,
        xblk=xblk,
        n_sub=n_sub,
        n_super=n_super,
        n_runs=n_runs,
        nchunk=nchunk,
        e_pad=e_pad,
        supers=supers,
    )
    return cfg, in_maps


def build_bass(cfg):
    F, H, NPAD, NPC = cfg["nfeat"], cfg["nhid"], cfg["npad"], cfg["npc"]
    XBLK = cfg["xblk"]
    KC = F // 128
    assert F % 128 == 0 and H == 128 and XBLK % 512 == 0 and NPAD % XBLK == 0

    nc = bacc.Bacc(
        "TRN2",
        target_bir_lowering=False,
        debug=False,
        enable_asserts=True,
        num_devices=cfg["n_cores"],
        num_swdge_queues=4,
    )
    f32, bf16, i16 = mybir.dt.float32, mybir.dt.bfloat16, mybir.dt.int16
    xT = nc.dram_tensor("xT", [F, NPAD], bf16, kind="ExternalInput")
    wmat = nc.dram_tensor("wmat", [F, H], bf16, kind="ExternalInput")
    bias = nc.dram_tensor("bias", [H, 1], f32, kind="ExternalInput")
    iota = nc.dram_tensor("iota", [128, SUB], bf16, kind="ExternalInput")
    idx = nc.dram_tensor("idx", [128, cfg["e_pad"] // 16], i16, kind="ExternalInput")
    dstloc = nc.dram_tensor("dstloc", [128, cfg["nchunk"]], bf16, kind="ExternalInput")
    ew = nc.dram_tensor("ew", [128, cfg["nchunk"]], bf16, kind="ExternalInput")
    support = nc.dram_tensor("support", [NPAD, H], bf16, kind="Internal")
    outT = nc.dram_tensor("outT", [H, NPC], f32, kind="ExternalOutput")

    AF = mybir.ActivationFunctionType
    with tile.TileContext(nc) as tc:
        with (
            tc.tile_pool(name="const", bufs=1) as cpool,
            tc.tile_pool(name="xt", bufs=3) as xpool,
            tc.tile_pool(name="sup", bufs=2) as spool,
            tc.tile_pool(name="gath", bufs=6) as gpool,
            tc.tile_pool(name="ind", bufs=6) as ipool,
            tc.tile_pool(name="meta", bufs=6) as mpool,
            tc.tile_pool(name="outb", bufs=2) as opool,
            tc.tile_pool(name="psum", bufs=8, space="PSUM") as ppool,
        ):
            nc.gpsimd.load_library(_mlp_lib)
            w_sb = cpool.tile([128, KC, H], bf16)
            nc.sync.dma_start(
                out=w_sb[:], in_=wmat.ap().rearrange("(c k) h -> k c h", k=128)
            )
            bias_sb = cpool.tile([H, 1], f32)
            nc.sync.dma_start(out=bias_sb[:], in_=bias.ap())
            iota_sb = cpool.tile([128, SUB], bf16)
            nc.sync.dma_start(out=iota_sb[:], in_=iota.ap())

            # ---- phase 1: support = x @ W (bf16), written to DRAM ----
            for blk in range(NPAD // XBLK):
                xts = []
                for kc in range(KC):
                    xt = xpool.tile([128, XBLK], bf16, tag=f"xt{kc}")
                    nc.sync.dma_start(
                        out=xt[:],
                        in_=xT.ap()[
                            kc * 128 : (kc + 1) * 128, blk * XBLK : (blk + 1) * XBLK
                        ],
                    )
                    xts.append(xt)
                st = spool.tile([128, XBLK], bf16)
                for i in range(XBLK // 128):
                    col = i * 128
                    ps = ppool.tile([128, 128], f32, tag="agg_ps")
                    for kc in range(KC):
                        nc.tensor.matmul(
                            ps[:],
                            xts[kc][:, col : col + 128],
                            w_sb[:, kc, :],
                            start=(kc == 0),
                            stop=(kc == KC - 1),
                        )
                    nc.scalar.activation(
                        out=st[:, col : col + 128], in_=ps[:], func=AF.Copy
                    )
                nc.sync.dma_start(
                    out=support.ap()[blk * XBLK : (blk + 1) * XBLK, :].rearrange(
                        "(i p) h -> p i h", p=128
                    ),
                    in_=st[:].rearrange("p (i h) -> p i h", h=H),
                )

            tc.strict_bb_all_engine_barrier()

            # ---- phase 2: gather + weighted-one-hot matmul segment sum ----
            # hoist num_idxs registers (few distinct values) so per-call
            # MOVEs don't WAR-serialize the gather stream
            lregs = {}
            for _sup in cfg["supers"]:
                for _call in _sup["calls"]:
                    _L = _call["n_chunks"] * 128
                    if _L not in lregs:
                        lregs[_L] = nc.gpsimd.to_reg(_L)
            qload = [0, 0, 0, 0]
            for S, sup in enumerate(cfg["supers"]):
                n_sub_here = _ceil_div(sup["w"], SUB)
                pss = [
                    ppool.tile([128, SUB], f32, name="agg_ps", tag="agg_ps")
                    for _ in range(n_sub_here)
                ]
                for call in sup["calls"]:
                    Gc = call["n_chunks"]
                    L = Gc * 128
                    idxt = mpool.tile([128, L // 16], i16, tag="idx")
                    c0 = call["chunk_off"] * 8  # idx plane col = chunk_off*128/16
                    nc.sync.dma_start(out=idxt[:], in_=idx.ap()[:, c0 : c0 + L // 16])
                    dlt = mpool.tile([128, Gc], bf16, tag="dl")
                    nc.sync.dma_start(
                        out=dlt[:],
                        in_=dstloc.ap()[:, call["chunk_off"] : call["chunk_off"] + Gc],
                    )
                    ewt = mpool.tile([128, Gc], bf16, tag="ew")
                    nc.sync.dma_start(
                        out=ewt[:],
                        in_=ew.ap()[:, call["chunk_off"] : call["chunk_off"] + Gc],
                    )
                    gt = gpool.tile([128, Gc, H], bf16)
                    gq = min(range(4), key=lambda k: qload[k])
                    qload[gq] += Gc
                    nc.gpsimd.dma_gather(
                        gt[:],
                        support.ap()[call["row0"] : call["row1"], :],
                        idxt[:],
                        L,
                        lregs[L],
                        H,
                        single_packet=False,
                        queue_num=gq,
                    )
                    ind = ipool.tile([128, Gc, SUB], bf16)
                    nc.vector.tensor_tensor(
                        out=ind[:],
                        in0=iota_sb[:][:, None, :].to_broadcast([128, Gc, SUB]),
                        in1=dlt[:][:, :, None].to_broadcast([128, Gc, SUB]),
                        op=mybir.AluOpType.is_equal,
                    )
                    nc.vector.tensor_tensor(
                        out=ind[:],
                        in0=ind[:],
                        in1=ewt[:][:, :, None].to_broadcast([128, Gc, SUB]),
                        op=mybir.AluOpType.mult,
                    )
                    for j in range(Gc):
                        so = call["sub_local"][j]
                        nc.tensor.matmul(
                            pss[so][:],
                            gt[:, j, :],
                            ind[:, j, :],
                            start=bool(call["start"][j]),
                            stop=bool(call["stop"][j]),
                        )
                ob = opool.tile([H, PSUM_COLS], f32)
                wS = sup["w"]
                for so in range(n_sub_here):
                    wcols = min(SUB, wS - so * SUB)
                    nc.scalar.activation(
                        out=ob[:, so * SUB : so * SUB + wcols],
                        in_=pss[so][:, :wcols],
                        func=AF.Relu,
                        bias=bias_sb[:],
                        scale=1.0,
                    )
                nc.sync.dma_start(
                    out=outT.ap()[:, S * PSUM_COLS : S * PSUM_COLS + wS],
                    in_=ob[:, :wS],
                )
    nc.compile()
    return nc


def kernel(x, edge_src, edge_dst, edge_w, W, b):
    x = np.asarray(x)
    cfg, in_maps = prepare(x, edge_src, edge_dst, edge_w, W, b)
    nc = build_bass(cfg)
    want_trace = bool(int(os.environ.get("GNN_TRACE", "0")))
    core_ids = list(range(cfg["n_cores"]))
    if want_trace:
        try:
            res = run_bass_kernel_spmd(nc, in_maps, core_ids=core_ids, trace=True)
        except Exception as e:
            print(f"traced run failed ({e}); retrying without trace")
            res = run_bass_kernel_spmd(nc, in_maps, core_ids=core_ids, trace=False)
    else:
        res = run_bass_kernel_spmd(nc, in_maps, core_ids=core_ids, trace=False)
    kernel.last_result = res
    out = np.concatenate([r["outT"].T for r in res.results], axis=0)
    return np.ascontiguousarray(out).astype(np.float32)


kernel.last_result = None

